# revision 12
# baseline (speedup 1.0000x reference)
"""BinLinear Trainium2 kernel.

Computes: out = input @ binarize(weight), where
  binarize(w) = +1 where tanh(w) >= 0 else -1  (== +1 where w >= 0 else -1)

Shapes (hardcoded per problem spec):
  input  [8192, 2048] f32
  weight [2048, 2048] f32
  out    [8192, 2048] f32

Strategy: data-parallel over the 8 NeuronCores — each core computes a
1024-row slice of the output.  Host-side prep:
  - binarize weight -> {-1,+1} bf16 (exact in bf16), k-tiled [16,128,2048]
  - transpose+cast input -> bf16 x^T shard [16,128,1024] per core so the
    contraction dim (k) lands on SBUF partitions with natural layout.
Device-side (per core): both operands fully SBUF-resident; 512 matmuls
(stationary = x^T tile [128k,128n], moving = w_b [128k,512m]) accumulating
over 16 k-tiles into PSUM, PSUM->SBUF copy on DVE, DMA out.
"""

import sys

for _p in ("/root/.axon_site/_ro/trn_rl_repo", "/opt/trn_rl_repo"):
    if _p not in sys.path:
        sys.path.append(_p)

import numpy as np
import ml_dtypes

import concourse.bass as bass
import concourse.bacc as bacc
import concourse.mybir as mybir
from concourse import tile
from concourse.bass_utils import run_bass_kernel_spmd

N, K, M = 8192, 2048, 2048
NCORES = 8
NC_ROWS = N // NCORES          # 1024 output rows per core
P = 128
KT = K // P                    # 16 k-tiles
NT = NC_ROWS // P              # 8 n-tiles per core
MCHUNK = 512                   # one PSUM bank of f32
NMC = M // MCHUNK              # 4 m-chunks
PAIR = 2                       # n-tiles processed kt-major together

_nc_cache = {}


def _build_nc():
    nc = bacc.Bacc(
        "TRN2",
        target_bir_lowering=False,
        debug=False,
        enable_asserts=False,
        num_devices=NCORES,
    )
    bf16 = mybir.dt.float16  # fp16: same PE rate as bf16, 8 more mantissa bits
    f32 = mybir.dt.float32

    xT_d = nc.dram_tensor("xT", [KT, P, NC_ROWS], bf16, kind="ExternalInput").ap()
    wb_d = nc.dram_tensor("wb", [KT, P, M], bf16, kind="ExternalInput").ap()
    out_d = nc.dram_tensor("out", [NC_ROWS, M], f32, kind="ExternalOutput").ap()

    # Phase plan: 4 "quad-half" phases, each = 4 n-tiles x 2 m-chunks
    # (8 PSUM banks), kt-major inside so the PE demand per arriving k-tile
    # (8 MMs ~ 1.7us) exceeds that k-tile's DMA time (~1.2us) from the
    # start -> PE-bound throughout. DMAs are emitted in exactly the order
    # phases consume them.
    NQ = 4                      # n-tiles per phase
    MH = 2                      # m-chunks per phase
    with tile.TileContext(nc) as tc:
        with (
            tc.tile_pool(name="xres", bufs=1) as xpool,
            tc.tile_pool(name="wres", bufs=1) as wpool,
            tc.tile_pool(name="ostage", bufs=4) as opool,
            tc.tile_pool(name="psum", bufs=1, space="PSUM") as ppool,
        ):
            xs = [
                xpool.tile([P, NC_ROWS], bf16, name=f"x{kt}", tag=f"x{kt}")
                for kt in range(KT)
            ]
            ws = [
                wpool.tile([P, M], bf16, name=f"w{kt}", tag=f"w{kt}")
                for kt in range(KT)
            ]
            phases = [
                (nq, mh) for nq in range(NT // NQ) for mh in range(NMC // MH)
            ]
            # DMA emission in phase-consumption order, chunks kept >=2KiB
            # per partition for descriptor efficiency:
            #   ph0 needs w[kt, mc01] + x[kt]; ph1 adds w[kt, mc23]
            MW = MH * MCHUNK  # 1024: weight m-half width
            for kt in range(KT):
                if kt == 0:
                    nc.sync.dma_start(out=ws[0][:, 0:MCHUNK], in_=wb_d[0][:, 0:MCHUNK])
                    nc.sync.dma_start(out=xs[0][:, 0 : 2 * P], in_=xT_d[0][:, 0 : 2 * P])
                    nc.sync.dma_start(out=ws[0][:, MCHUNK:MW], in_=wb_d[0][:, MCHUNK:MW])
                    nc.sync.dma_start(out=xs[0][:, 2 * P :], in_=xT_d[0][:, 2 * P :])
                    continue
                nc.sync.dma_start(out=ws[kt][:, 0:MW], in_=wb_d[kt][:, 0:MW])
                nc.sync.dma_start(out=xs[kt][:], in_=xT_d[kt])
            for kt in range(KT):
                nc.sync.dma_start(out=ws[kt][:, MW:M], in_=wb_d[kt][:, MW:M])

            # PE pre-warm: dummy matmuls on zeroed scratch while input
            # DMAs stream, so HAM un-throttles (1.2->2.4GHz) before the
            # real MM stream begins. The warm psum tile shares slot
            # ps0_0; Tile serializes the slot handoff.
            xsc = xpool.tile([P, P], bf16, name="xsc", tag="xsc")
            wsc = wpool.tile([P, MCHUNK], bf16, name="wsc", tag="wsc")
            nc.gpsimd.memset(xsc[:], 0.0)
            nc.gpsimd.memset(wsc[:], 0.0)
            wm = ppool.tile([P, MCHUNK], f32, name="warm", tag="ps0_0")
            for _ in range(48):
                nc.tensor.matmul(wm[:], xsc[:], wsc[:], start=True, stop=True)

            def emit_store(nt, mc, ps, idx):
                so = opool.tile([P, MCHUNK], f32, name=f"so{nt}_{mc}", tag="so")
                if idx % 2 == 0:
                    nc.vector.tensor_copy(so[:], ps[:])
                else:
                    nc.scalar.copy(so[:], ps[:])
                nc.scalar.dma_start(
                    out=out_d[
                        nt * P : (nt + 1) * P, mc * MCHUNK : (mc + 1) * MCHUNK
                    ],
                    in_=so[:],
                )

            for pi, (nq, mh) in enumerate(phases):
                nts = list(range(nq * NQ, (nq + 1) * NQ))
                mcs = list(range(mh * MH, (mh + 1) * MH))
                pss = {
                    (nt, mc): ppool.tile(
                        [P, MCHUNK],
                        f32,
                        name=f"ps{nt}_{mc}",
                        tag=f"ps{nt % NQ}_{mc % MH}",
                    )
                    for nt in nts
                    for mc in mcs
                }
                if pi < 2:
                    # streaming phases: kt-major so each arriving k-tile
                    # feeds 8 MMs
                    for kt in range(KT):
                        for nt in nts:
                            lhsT = xs[kt][:, nt * P : (nt + 1) * P]
                            for mc in mcs:
                                nc.tensor.matmul(
                                    pss[(nt, mc)][:],
                                    lhsT,
                                    ws[kt][:, mc * MCHUNK : (mc + 1) * MCHUNK],
                                    start=(kt == 0),
                                    stop=(kt == KT - 1),
                                )
                    for i, nt in enumerate(nts):
                        for j, mc in enumerate(mcs):
                            emit_store(nt, mc, pss[(nt, mc)], i * MH + j)
                else:
                    # resident phases: nt-major so stores overlap the
                    # remaining MM stream (cuts the kernel tail)
                    for i, nt in enumerate(nts):
                        for kt in range(KT):
                            lhsT = xs[kt][:, nt * P : (nt + 1) * P]
                            for mc in mcs:
                                nc.tensor.matmul(
                                    pss[(nt, mc)][:],
                                    lhsT,
                                    ws[kt][:, mc * MCHUNK : (mc + 1) * MCHUNK],
                                    start=(kt == 0),
                                    stop=(kt == KT - 1),
                                )
                        for j, mc in enumerate(mcs):
                            emit_store(nt, mc, pss[(nt, mc)], i * MH + j)
    nc.compile()
    return nc


def _get_nc():
    if "nc" not in _nc_cache:
        _nc_cache["nc"] = _build_nc()
    return _nc_cache["nc"]


def _prep_inputs(input, weight):
    input = np.asarray(input, dtype=np.float32)
    weight = np.asarray(weight, dtype=np.float32)
    # binarize: sign of tanh(w) == sign of w; w==0 -> +1 (matches >= 0)
    wb = np.where(weight >= 0.0, np.float32(1.0), np.float32(-1.0))
    wb_t = np.ascontiguousarray(
        wb.astype(np.float16).reshape(KT, P, M)
    )
    xT = input.astype(np.float16).T.reshape(KT, P, N)
    in_maps = []
    for c in range(NCORES):
        x_shard = np.ascontiguousarray(xT[:, :, c * NC_ROWS : (c + 1) * NC_ROWS])
        in_maps.append({"xT": x_shard, "wb": wb_t})
    return in_maps


def _run(in_maps, trace=False):
    nc = _get_nc()
    return run_bass_kernel_spmd(nc, in_maps, list(range(NCORES)), trace=trace)


def kernel(input, weight):
    in_maps = _prep_inputs(input, weight)
    res = _run(in_maps, trace=False)
    return np.concatenate([r["out"] for r in res.results], axis=0)


def bench(input, weight):
    """Correctness + HW-profiled run. Returns (out, exec_time_ns)."""
    in_maps = _prep_inputs(input, weight)
    res = _run(in_maps, trace=True)
    out = np.concatenate([r["out"] for r in res.results], axis=0)
    return out, res.exec_time_ns


# revision 13
# speedup vs baseline: 1.0266x; 1.0266x over previous
"""BinLinear Trainium2 kernel.

Computes: out = input @ binarize(weight), where
  binarize(w) = +1 where tanh(w) >= 0 else -1  (== +1 where w >= 0 else -1)

Shapes (hardcoded per problem spec):
  input  [8192, 2048] f32
  weight [2048, 2048] f32
  out    [8192, 2048] f32

Strategy: data-parallel over the 8 NeuronCores — each core computes a
1024-row slice of the output.  Host-side prep:
  - binarize weight -> {-1,+1} bf16 (exact in bf16), k-tiled [16,128,2048]
  - transpose+cast input -> bf16 x^T shard [16,128,1024] per core so the
    contraction dim (k) lands on SBUF partitions with natural layout.
Device-side (per core): both operands fully SBUF-resident; 512 matmuls
(stationary = x^T tile [128k,128n], moving = w_b [128k,512m]) accumulating
over 16 k-tiles into PSUM, PSUM->SBUF copy on DVE, DMA out.
"""

import sys

for _p in ("/root/.axon_site/_ro/trn_rl_repo", "/opt/trn_rl_repo"):
    if _p not in sys.path:
        sys.path.append(_p)

import numpy as np
import ml_dtypes

import concourse.bass as bass
import concourse.bacc as bacc
import concourse.mybir as mybir
from concourse import tile
from concourse.bass_utils import run_bass_kernel_spmd

N, K, M = 8192, 2048, 2048
NCORES = 8
NC_ROWS = N // NCORES          # 1024 output rows per core
P = 128
KT = K // P                    # 16 k-tiles
NT = NC_ROWS // P              # 8 n-tiles per core
MCHUNK = 512                   # one PSUM bank of f32
NMC = M // MCHUNK              # 4 m-chunks
PAIR = 2                       # n-tiles processed kt-major together

_nc_cache = {}


def _build_nc():
    nc = bacc.Bacc(
        "TRN2",
        target_bir_lowering=False,
        debug=False,
        enable_asserts=False,
        num_devices=NCORES,
    )
    bf16 = mybir.dt.float16  # fp16: same PE rate as bf16, 8 more mantissa bits
    f32 = mybir.dt.float32

    xT_d = nc.dram_tensor("xT", [KT, P, NC_ROWS], bf16, kind="ExternalInput").ap()
    wb_d = nc.dram_tensor("wb", [KT, P, M], bf16, kind="ExternalInput").ap()
    out_d = nc.dram_tensor("out", [NC_ROWS, M], f32, kind="ExternalOutput").ap()

    # Phase plan: 4 "quad-half" phases, each = 4 n-tiles x 2 m-chunks
    # (8 PSUM banks), kt-major inside so the PE demand per arriving k-tile
    # (8 MMs ~ 1.7us) exceeds that k-tile's DMA time (~1.2us) from the
    # start -> PE-bound throughout. DMAs are emitted in exactly the order
    # phases consume them.
    NQ = 4                      # n-tiles per phase
    MH = 2                      # m-chunks per phase
    with tile.TileContext(nc) as tc:
        with (
            tc.tile_pool(name="xres", bufs=1) as xpool,
            tc.tile_pool(name="wres", bufs=1) as wpool,
            tc.tile_pool(name="ostage", bufs=4) as opool,
            tc.tile_pool(name="psum", bufs=1, space="PSUM") as ppool,
        ):
            xs = [
                xpool.tile([P, NC_ROWS], bf16, name=f"x{kt}", tag=f"x{kt}")
                for kt in range(KT)
            ]
            ws = [
                wpool.tile([P, M], bf16, name=f"w{kt}", tag=f"w{kt}")
                for kt in range(KT)
            ]
            phases = [
                (nq, mh) for nq in range(NT // NQ) for mh in range(NMC // MH)
            ]
            # DMA emission in phase-consumption order, chunks kept >=2KiB
            # per partition for descriptor efficiency:
            #   ph0 needs w[kt, mc01] + x[kt]; ph1 adds w[kt, mc23]
            MW = MH * MCHUNK  # 1024: weight m-half width
            for kt in range(KT):
                if kt == 0:
                    nc.sync.dma_start(out=ws[0][:, 0:MCHUNK], in_=wb_d[0][:, 0:MCHUNK])
                    nc.sync.dma_start(out=xs[0][:, 0 : 2 * P], in_=xT_d[0][:, 0 : 2 * P])
                    nc.sync.dma_start(out=ws[0][:, MCHUNK:MW], in_=wb_d[0][:, MCHUNK:MW])
                    nc.sync.dma_start(out=xs[0][:, 2 * P :], in_=xT_d[0][:, 2 * P :])
                    continue
                nc.sync.dma_start(out=ws[kt][:, 0:MW], in_=wb_d[kt][:, 0:MW])
                nc.sync.dma_start(out=xs[kt][:], in_=xT_d[kt])
            for kt in range(KT):
                nc.sync.dma_start(out=ws[kt][:, MW:M], in_=wb_d[kt][:, MW:M])

            # PE pre-warm: dummy matmuls on zeroed scratch while input
            # DMAs stream, so HAM un-throttles (1.2->2.4GHz) before the
            # real MM stream begins. The warm psum tile shares slot
            # ps0_0; Tile serializes the slot handoff.
            xsc = xpool.tile([P, P], bf16, name="xsc", tag="xsc")
            wsc = wpool.tile([P, MCHUNK], bf16, name="wsc", tag="wsc")
            nc.gpsimd.memset(xsc[:], 0.0)
            nc.gpsimd.memset(wsc[:], 0.0)
            wm = ppool.tile([P, MCHUNK], f32, name="warm", tag="ps0_0")
            for _ in range(16):
                nc.tensor.matmul(wm[:], xsc[:], wsc[:], start=True, stop=True)

            def emit_store(nt, mc, ps, idx):
                so = opool.tile([P, MCHUNK], f32, name=f"so{nt}_{mc}", tag="so")
                if idx % 2 == 0:
                    nc.vector.tensor_copy(so[:], ps[:])
                else:
                    nc.scalar.copy(so[:], ps[:])
                nc.scalar.dma_start(
                    out=out_d[
                        nt * P : (nt + 1) * P, mc * MCHUNK : (mc + 1) * MCHUNK
                    ],
                    in_=so[:],
                )

            for pi, (nq, mh) in enumerate(phases):
                nts = list(range(nq * NQ, (nq + 1) * NQ))
                mcs = list(range(mh * MH, (mh + 1) * MH))
                pss = {
                    (nt, mc): ppool.tile(
                        [P, MCHUNK],
                        f32,
                        name=f"ps{nt}_{mc}",
                        tag=f"ps{nt % NQ}_{mc % MH}",
                    )
                    for nt in nts
                    for mc in mcs
                }
                if pi < 2:
                    # streaming phases: kt-major so each arriving k-tile
                    # feeds 8 MMs
                    for kt in range(KT):
                        for nt in nts:
                            lhsT = xs[kt][:, nt * P : (nt + 1) * P]
                            for mc in mcs:
                                nc.tensor.matmul(
                                    pss[(nt, mc)][:],
                                    lhsT,
                                    ws[kt][:, mc * MCHUNK : (mc + 1) * MCHUNK],
                                    start=(kt == 0),
                                    stop=(kt == KT - 1),
                                )
                    for i, nt in enumerate(nts):
                        for j, mc in enumerate(mcs):
                            emit_store(nt, mc, pss[(nt, mc)], i * MH + j)
                else:
                    # resident phases: nt-major so stores overlap the
                    # remaining MM stream (cuts the kernel tail)
                    for i, nt in enumerate(nts):
                        for kt in range(KT):
                            lhsT = xs[kt][:, nt * P : (nt + 1) * P]
                            for mc in mcs:
                                nc.tensor.matmul(
                                    pss[(nt, mc)][:],
                                    lhsT,
                                    ws[kt][:, mc * MCHUNK : (mc + 1) * MCHUNK],
                                    start=(kt == 0),
                                    stop=(kt == KT - 1),
                                )
                        for j, mc in enumerate(mcs):
                            emit_store(nt, mc, pss[(nt, mc)], i * MH + j)
    nc.compile()
    return nc


def _get_nc():
    if "nc" not in _nc_cache:
        _nc_cache["nc"] = _build_nc()
    return _nc_cache["nc"]


def _prep_inputs(input, weight):
    input = np.asarray(input, dtype=np.float32)
    weight = np.asarray(weight, dtype=np.float32)
    # binarize: sign of tanh(w) == sign of w; w==0 -> +1 (matches >= 0)
    wb = np.where(weight >= 0.0, np.float32(1.0), np.float32(-1.0))
    wb_t = np.ascontiguousarray(
        wb.astype(np.float16).reshape(KT, P, M)
    )
    xT = input.astype(np.float16).T.reshape(KT, P, N)
    in_maps = []
    for c in range(NCORES):
        x_shard = np.ascontiguousarray(xT[:, :, c * NC_ROWS : (c + 1) * NC_ROWS])
        in_maps.append({"xT": x_shard, "wb": wb_t})
    return in_maps


def _run(in_maps, trace=False):
    nc = _get_nc()
    return run_bass_kernel_spmd(nc, in_maps, list(range(NCORES)), trace=trace)


def kernel(input, weight):
    in_maps = _prep_inputs(input, weight)
    res = _run(in_maps, trace=False)
    return np.concatenate([r["out"] for r in res.results], axis=0)


def bench(input, weight):
    """Correctness + HW-profiled run. Returns (out, exec_time_ns)."""
    in_maps = _prep_inputs(input, weight)
    res = _run(in_maps, trace=True)
    out = np.concatenate([r["out"] for r in res.results], axis=0)
    return out, res.exec_time_ns


# revision 14
# speedup vs baseline: 1.0561x; 1.0288x over previous
"""BinLinear Trainium2 kernel.

Computes: out = input @ binarize(weight), where
  binarize(w) = +1 where tanh(w) >= 0 else -1  (== +1 where w >= 0 else -1)

Shapes (hardcoded per problem spec):
  input  [8192, 2048] f32
  weight [2048, 2048] f32
  out    [8192, 2048] f32

Strategy: data-parallel over the 8 NeuronCores — each core computes a
1024-row slice of the output.  Host-side prep:
  - binarize weight -> {-1,+1} bf16 (exact in bf16), k-tiled [16,128,2048]
  - transpose+cast input -> bf16 x^T shard [16,128,1024] per core so the
    contraction dim (k) lands on SBUF partitions with natural layout.
Device-side (per core): both operands fully SBUF-resident; 512 matmuls
(stationary = x^T tile [128k,128n], moving = w_b [128k,512m]) accumulating
over 16 k-tiles into PSUM, PSUM->SBUF copy on DVE, DMA out.
"""

import sys

for _p in ("/root/.axon_site/_ro/trn_rl_repo", "/opt/trn_rl_repo"):
    if _p not in sys.path:
        sys.path.append(_p)

import numpy as np
import ml_dtypes

import concourse.bass as bass
import concourse.bacc as bacc
import concourse.mybir as mybir
from concourse import tile
from concourse.bass_utils import run_bass_kernel_spmd

N, K, M = 8192, 2048, 2048
NCORES = 8
NC_ROWS = N // NCORES          # 1024 output rows per core
P = 128
KT = K // P                    # 16 k-tiles
NT = NC_ROWS // P              # 8 n-tiles per core
MCHUNK = 512                   # one PSUM bank of f32
NMC = M // MCHUNK              # 4 m-chunks
PAIR = 2                       # n-tiles processed kt-major together

_nc_cache = {}


def _build_nc():
    nc = bacc.Bacc(
        "TRN2",
        target_bir_lowering=False,
        debug=False,
        enable_asserts=False,
        num_devices=NCORES,
    )
    bf16 = mybir.dt.float16  # fp16: same PE rate as bf16, 8 more mantissa bits
    f32 = mybir.dt.float32

    xT_d = nc.dram_tensor("xT", [KT, P, NC_ROWS], bf16, kind="ExternalInput").ap()
    wb_d = nc.dram_tensor("wb", [KT, P, M], bf16, kind="ExternalInput").ap()
    out_d = nc.dram_tensor("out", [NC_ROWS, M], f32, kind="ExternalOutput").ap()

    # Phase plan: 4 "quad-half" phases, each = 4 n-tiles x 2 m-chunks
    # (8 PSUM banks), kt-major inside so the PE demand per arriving k-tile
    # (8 MMs ~ 1.7us) exceeds that k-tile's DMA time (~1.2us) from the
    # start -> PE-bound throughout. DMAs are emitted in exactly the order
    # phases consume them.
    NQ = 4                      # n-tiles per phase
    MH = 2                      # m-chunks per phase
    with tile.TileContext(nc) as tc:
        with (
            tc.tile_pool(name="xres", bufs=1) as xpool,
            tc.tile_pool(name="wres", bufs=1) as wpool,
            tc.tile_pool(name="ostage", bufs=4) as opool,
            tc.tile_pool(name="psum", bufs=1, space="PSUM") as ppool,
        ):
            xs = [
                xpool.tile([P, NC_ROWS], bf16, name=f"x{kt}", tag=f"x{kt}")
                for kt in range(KT)
            ]
            ws = [
                wpool.tile([P, M], bf16, name=f"w{kt}", tag=f"w{kt}")
                for kt in range(KT)
            ]
            phases = [
                (nq, mh) for nq in range(NT // NQ) for mh in range(NMC // MH)
            ]
            # DMA emission in phase-consumption order, chunks kept >=2KiB
            # per partition for descriptor efficiency:
            #   ph0 needs w[kt, mc01] + x[kt]; ph1 adds w[kt, mc23]
            MW = MH * MCHUNK  # 1024: weight m-half width
            for kt in range(KT):
                nc.sync.dma_start(out=ws[kt][:, 0:MW], in_=wb_d[kt][:, 0:MW])
                nc.sync.dma_start(out=xs[kt][:], in_=xT_d[kt])
            for kt in range(KT):
                nc.sync.dma_start(out=ws[kt][:, MW:M], in_=wb_d[kt][:, MW:M])

            # PE pre-warm: dummy matmuls on zeroed scratch while input
            # DMAs stream, so HAM un-throttles (1.2->2.4GHz) before the
            # real MM stream begins.
            xsc = xpool.tile([P, P], bf16, name="xsc", tag="xsc")
            wsc = wpool.tile([P, MCHUNK], bf16, name="wsc", tag="wsc")
            nc.gpsimd.memset(xsc[:], 0.0)
            nc.gpsimd.memset(wsc[:], 0.0)
            wm = ppool.tile([P, MCHUNK], f32, name="warm", tag="ps0_0")
            for _ in range(16):
                nc.tensor.matmul(wm[:], xsc[:], wsc[:], start=True, stop=True)

            def emit_store(nt, mc, ps, idx):
                so = opool.tile([P, MCHUNK], f32, name=f"so{nt}_{mc}", tag="so")
                if idx % 2 == 0:
                    nc.vector.tensor_copy(so[:], ps[:])
                else:
                    nc.scalar.copy(so[:], ps[:])
                nc.sync.dma_start(
                    out=out_d[
                        nt * P : (nt + 1) * P, mc * MCHUNK : (mc + 1) * MCHUNK
                    ],
                    in_=so[:],
                )

            for pi, (nq, mh) in enumerate(phases):
                nts = list(range(nq * NQ, (nq + 1) * NQ))
                mcs = list(range(mh * MH, (mh + 1) * MH))
                pss = {
                    (nt, mc): ppool.tile(
                        [P, MCHUNK],
                        f32,
                        name=f"ps{nt}_{mc}",
                        tag=f"ps{nt % NQ}_{mc % MH}",
                    )
                    for nt in nts
                    for mc in mcs
                }
                if pi < 2:
                    # streaming phases: kt-major so each arriving k-tile
                    # feeds 8 MMs
                    for kt in range(KT):
                        for nt in nts:
                            lhsT = xs[kt][:, nt * P : (nt + 1) * P]
                            for mc in mcs:
                                nc.tensor.matmul(
                                    pss[(nt, mc)][:],
                                    lhsT,
                                    ws[kt][:, mc * MCHUNK : (mc + 1) * MCHUNK],
                                    start=(kt == 0),
                                    stop=(kt == KT - 1),
                                )
                    for i, nt in enumerate(nts):
                        for j, mc in enumerate(mcs):
                            emit_store(nt, mc, pss[(nt, mc)], i * MH + j)
                else:
                    # resident phases: nt-major so stores overlap the
                    # remaining MM stream (cuts the kernel tail)
                    for i, nt in enumerate(nts):
                        for kt in range(KT):
                            lhsT = xs[kt][:, nt * P : (nt + 1) * P]
                            for mc in mcs:
                                nc.tensor.matmul(
                                    pss[(nt, mc)][:],
                                    lhsT,
                                    ws[kt][:, mc * MCHUNK : (mc + 1) * MCHUNK],
                                    start=(kt == 0),
                                    stop=(kt == KT - 1),
                                )
                        for j, mc in enumerate(mcs):
                            emit_store(nt, mc, pss[(nt, mc)], i * MH + j)
    nc.compile()
    return nc


def _get_nc():
    if "nc" not in _nc_cache:
        _nc_cache["nc"] = _build_nc()
    return _nc_cache["nc"]


def _prep_inputs(input, weight):
    input = np.asarray(input, dtype=np.float32)
    weight = np.asarray(weight, dtype=np.float32)
    # binarize: sign of tanh(w) == sign of w; w==0 -> +1 (matches >= 0)
    wb = np.where(weight >= 0.0, np.float32(1.0), np.float32(-1.0))
    wb_t = np.ascontiguousarray(
        wb.astype(np.float16).reshape(KT, P, M)
    )
    xT = input.astype(np.float16).T.reshape(KT, P, N)
    in_maps = []
    for c in range(NCORES):
        x_shard = np.ascontiguousarray(xT[:, :, c * NC_ROWS : (c + 1) * NC_ROWS])
        in_maps.append({"xT": x_shard, "wb": wb_t})
    return in_maps


def _run(in_maps, trace=False):
    nc = _get_nc()
    return run_bass_kernel_spmd(nc, in_maps, list(range(NCORES)), trace=trace)


def kernel(input, weight):
    in_maps = _prep_inputs(input, weight)
    res = _run(in_maps, trace=False)
    return np.concatenate([r["out"] for r in res.results], axis=0)


def bench(input, weight):
    """Correctness + HW-profiled run. Returns (out, exec_time_ns)."""
    in_maps = _prep_inputs(input, weight)
    res = _run(in_maps, trace=True)
    out = np.concatenate([r["out"] for r in res.results], axis=0)
    return out, res.exec_time_ns


# revision 15
# speedup vs baseline: 1.0574x; 1.0012x over previous
"""BinLinear Trainium2 kernel.

Computes: out = input @ binarize(weight), where
  binarize(w) = +1 where tanh(w) >= 0 else -1  (== +1 where w >= 0 else -1)

Shapes (hardcoded per problem spec):
  input  [8192, 2048] f32
  weight [2048, 2048] f32
  out    [8192, 2048] f32

Strategy: data-parallel over the 8 NeuronCores — each core computes a
1024-row slice of the output.  Host-side prep:
  - binarize weight -> {-1,+1} bf16 (exact in bf16), k-tiled [16,128,2048]
  - transpose+cast input -> bf16 x^T shard [16,128,1024] per core so the
    contraction dim (k) lands on SBUF partitions with natural layout.
Device-side (per core): both operands fully SBUF-resident; 512 matmuls
(stationary = x^T tile [128k,128n], moving = w_b [128k,512m]) accumulating
over 16 k-tiles into PSUM, PSUM->SBUF copy on DVE, DMA out.
"""

import sys

for _p in ("/root/.axon_site/_ro/trn_rl_repo", "/opt/trn_rl_repo"):
    if _p not in sys.path:
        sys.path.append(_p)

import numpy as np
import ml_dtypes

import concourse.bass as bass
import concourse.bacc as bacc
import concourse.mybir as mybir
from concourse import tile
from concourse.bass_utils import run_bass_kernel_spmd

N, K, M = 8192, 2048, 2048
NCORES = 8
NC_ROWS = N // NCORES          # 1024 output rows per core
P = 128
KT = K // P                    # 16 k-tiles
NT = NC_ROWS // P              # 8 n-tiles per core
MCHUNK = 512                   # one PSUM bank of f32
NMC = M // MCHUNK              # 4 m-chunks
PAIR = 2                       # n-tiles processed kt-major together

_nc_cache = {}


def _build_nc():
    nc = bacc.Bacc(
        "TRN2",
        target_bir_lowering=False,
        debug=False,
        enable_asserts=False,
        num_devices=NCORES,
    )
    bf16 = mybir.dt.float16  # fp16: same PE rate as bf16, 8 more mantissa bits
    f32 = mybir.dt.float32

    xT_d = nc.dram_tensor("xT", [KT, P, NC_ROWS], bf16, kind="ExternalInput").ap()
    wb_d = nc.dram_tensor("wb", [KT, P, M], bf16, kind="ExternalInput").ap()
    out_d = nc.dram_tensor("out", [NC_ROWS, M], f32, kind="ExternalOutput").ap()

    # Phase plan: 4 "quad-half" phases, each = 4 n-tiles x 2 m-chunks
    # (8 PSUM banks), kt-major inside so the PE demand per arriving k-tile
    # (8 MMs ~ 1.7us) exceeds that k-tile's DMA time (~1.2us) from the
    # start -> PE-bound throughout. DMAs are emitted in exactly the order
    # phases consume them.
    NQ = 4                      # n-tiles per phase
    MH = 2                      # m-chunks per phase
    with tile.TileContext(nc) as tc:
        with (
            tc.tile_pool(name="xres", bufs=1) as xpool,
            tc.tile_pool(name="wres", bufs=1) as wpool,
            tc.tile_pool(name="ostage", bufs=4) as opool,
            tc.tile_pool(name="psum", bufs=1, space="PSUM") as ppool,
        ):
            xs = [
                xpool.tile([P, NC_ROWS], bf16, name=f"x{kt}", tag=f"x{kt}")
                for kt in range(KT)
            ]
            ws = [
                wpool.tile([P, M], bf16, name=f"w{kt}", tag=f"w{kt}")
                for kt in range(KT)
            ]
            phases = [
                (nq, mh) for nq in range(NT // NQ) for mh in range(NMC // MH)
            ]
            # DMA emission in phase-consumption order, chunks kept >=2KiB
            # per partition for descriptor efficiency:
            #   ph0 needs w[kt, mc01] + x[kt]; ph1 adds w[kt, mc23]
            MW = MH * MCHUNK  # 1024: weight m-half width
            for kt in range(KT):
                if kt == 0:
                    # finer first pieces so the first real MM starts earlier
                    nc.sync.dma_start(out=ws[0][:, 0:MCHUNK], in_=wb_d[0][:, 0:MCHUNK])
                    nc.sync.dma_start(out=xs[0][:, 0 : 2 * P], in_=xT_d[0][:, 0 : 2 * P])
                    nc.sync.dma_start(out=ws[0][:, MCHUNK:MW], in_=wb_d[0][:, MCHUNK:MW])
                    nc.sync.dma_start(out=xs[0][:, 2 * P :], in_=xT_d[0][:, 2 * P :])
                    continue
                nc.sync.dma_start(out=ws[kt][:, 0:MW], in_=wb_d[kt][:, 0:MW])
                nc.sync.dma_start(out=xs[kt][:], in_=xT_d[kt])
            for kt in range(KT):
                nc.sync.dma_start(out=ws[kt][:, MW:M], in_=wb_d[kt][:, MW:M])

            # PE pre-warm: dummy matmuls on zeroed scratch while input
            # DMAs stream, so HAM un-throttles (1.2->2.4GHz) before the
            # real MM stream begins.
            xsc = xpool.tile([P, P], bf16, name="xsc", tag="xsc")
            wsc = wpool.tile([P, MCHUNK], bf16, name="wsc", tag="wsc")
            nc.gpsimd.memset(xsc[:], 0.0)
            nc.gpsimd.memset(wsc[:], 0.0)
            wm = ppool.tile([P, MCHUNK], f32, name="warm", tag="ps0_0")
            for _ in range(16):
                nc.tensor.matmul(wm[:], xsc[:], wsc[:], start=True, stop=True)

            def emit_store(nt, mc, ps, idx):
                so = opool.tile([P, MCHUNK], f32, name=f"so{nt}_{mc}", tag="so")
                if idx % 2 == 0:
                    nc.vector.tensor_copy(so[:], ps[:])
                else:
                    nc.scalar.copy(so[:], ps[:])
                nc.sync.dma_start(
                    out=out_d[
                        nt * P : (nt + 1) * P, mc * MCHUNK : (mc + 1) * MCHUNK
                    ],
                    in_=so[:],
                )

            for pi, (nq, mh) in enumerate(phases):
                nts = list(range(nq * NQ, (nq + 1) * NQ))
                mcs = list(range(mh * MH, (mh + 1) * MH))
                pss = {
                    (nt, mc): ppool.tile(
                        [P, MCHUNK],
                        f32,
                        name=f"ps{nt}_{mc}",
                        tag=f"ps{nt % NQ}_{mc % MH}",
                    )
                    for nt in nts
                    for mc in mcs
                }
                if pi < 2:
                    # streaming phases: kt-major so each arriving k-tile
                    # feeds 8 MMs
                    for kt in range(KT):
                        for nt in nts:
                            lhsT = xs[kt][:, nt * P : (nt + 1) * P]
                            for mc in mcs:
                                nc.tensor.matmul(
                                    pss[(nt, mc)][:],
                                    lhsT,
                                    ws[kt][:, mc * MCHUNK : (mc + 1) * MCHUNK],
                                    start=(kt == 0),
                                    stop=(kt == KT - 1),
                                )
                    for i, nt in enumerate(nts):
                        for j, mc in enumerate(mcs):
                            emit_store(nt, mc, pss[(nt, mc)], i * MH + j)
                else:
                    # resident phases: nt-major so stores overlap the
                    # remaining MM stream (cuts the kernel tail)
                    for i, nt in enumerate(nts):
                        for kt in range(KT):
                            lhsT = xs[kt][:, nt * P : (nt + 1) * P]
                            for mc in mcs:
                                nc.tensor.matmul(
                                    pss[(nt, mc)][:],
                                    lhsT,
                                    ws[kt][:, mc * MCHUNK : (mc + 1) * MCHUNK],
                                    start=(kt == 0),
                                    stop=(kt == KT - 1),
                                )
                        for j, mc in enumerate(mcs):
                            emit_store(nt, mc, pss[(nt, mc)], i * MH + j)
    nc.compile()
    return nc


def _get_nc():
    if "nc" not in _nc_cache:
        _nc_cache["nc"] = _build_nc()
    return _nc_cache["nc"]


def _prep_inputs(input, weight):
    input = np.asarray(input, dtype=np.float32)
    weight = np.asarray(weight, dtype=np.float32)
    # binarize: sign of tanh(w) == sign of w; w==0 -> +1 (matches >= 0)
    wb = np.where(weight >= 0.0, np.float32(1.0), np.float32(-1.0))
    wb_t = np.ascontiguousarray(
        wb.astype(np.float16).reshape(KT, P, M)
    )
    xT = input.astype(np.float16).T.reshape(KT, P, N)
    in_maps = []
    for c in range(NCORES):
        x_shard = np.ascontiguousarray(xT[:, :, c * NC_ROWS : (c + 1) * NC_ROWS])
        in_maps.append({"xT": x_shard, "wb": wb_t})
    return in_maps


def _run(in_maps, trace=False):
    nc = _get_nc()
    return run_bass_kernel_spmd(nc, in_maps, list(range(NCORES)), trace=trace)


def kernel(input, weight):
    in_maps = _prep_inputs(input, weight)
    res = _run(in_maps, trace=False)
    return np.concatenate([r["out"] for r in res.results], axis=0)


def bench(input, weight):
    """Correctness + HW-profiled run. Returns (out, exec_time_ns)."""
    in_maps = _prep_inputs(input, weight)
    res = _run(in_maps, trace=True)
    out = np.concatenate([r["out"] for r in res.results], axis=0)
    return out, res.exec_time_ns
